# revision 5
# baseline (speedup 1.0000x reference)
"""Causal multi-head attention (B=4, T=2048, H=16, hs=64, D=1024) on 8
Trainium2 NeuronCores — bf16 data path + pair-level software pipelining.

Sharding: tensor-parallel over heads — each core computes 2 heads'
Q/K/V projections + attention, then a partial output projection
(y_partial = O_2h @ Wo[:, core_cols].T).  Host sums the 8 partials and
adds the bias.

Engine schedule (per-engine programs execute in order, so emission order
IS the schedule):
  - attention is emitted pair-by-pair with one pair of lookahead: the
    S-matmuls of pair i+1 (and their exp on ACT) are emitted before the
    AV-matmuls of pair i, so the PE computes S(i+1) while ACT exps pair i
    instead of stalling in-order on AV(i).
  - next batch's projection work is split into micro-units (4-matmul
    half-chains, PSUM-evac copies, V-transpose groups) and dribbled
    between attention quanta at a fixed pace, giving the PE independent
    fill work for the exp-latency gaps.
  - each query-block's output projection is deferred a few quanta into
    the following block so the PE never waits in-order on the
    reciprocal->broadcast->normalise chain (DVE/GPSIMD).
All matmul operands are bf16 (PSUM stays fp32); causal masks are bf16
multiplies in the DVE's 4x packed mode; x and y cross HBM as bf16.
"""

from contextlib import ExitStack

import numpy as np

import concourse.mybir as mybir
import concourse.tile as tile
from concourse import bacc

F32 = mybir.dt.float32
BF16 = mybir.dt.bfloat16
EXP = mybir.ActivationFunctionType.Exp

# problem shape (hardcoded per harness contract)
B, T, D, H, HS = 4, 2048, 1024, 16, 64
N_CORES = 8
HPC = H // N_CORES          # heads per core = 2
QB = 512                    # query block (matmul moving dim)
KC = 128                    # key chunk (partition dim)
SCALE = HS ** -0.5


def build_nc(b=B, t=T, d=D, hpc=HPC, loop_n=1, pace=0.82, defer_out=3):
    """Build the per-core program. All cores run the same NEFF; per-core
    data (weight slices) comes in through the input tensors."""
    n_dc = d // 128           # D chunks (contraction for projections)
    n_qb = t // QB            # query blocks
    n_kc = t // KC            # key chunks
    mh = 64 * hpc             # packed head width (=128 for hpc=2)

    nc = bacc.Bacc("TRN2", target_bir_lowering=False, debug=False)

    xT = nc.dram_tensor("xT", [b, d, t], BF16, kind="ExternalInput").ap()
    wq = nc.dram_tensor("wq", [d, mh], BF16, kind="ExternalInput").ap()
    wk = nc.dram_tensor("wk", [d, mh], BF16, kind="ExternalInput").ap()
    wv = nc.dram_tensor("wv", [d, mh], BF16, kind="ExternalInput").ap()
    woT = nc.dram_tensor("woT", [mh, d], BF16, kind="ExternalInput").ap()
    masks = nc.dram_tensor("masks", [2, KC, 2 * QB], BF16, kind="ExternalInput").ap()
    ident = nc.dram_tensor("ident", [128, 64], F32, kind="ExternalInput").ap()
    y = nc.dram_tensor("y", [b, t, d], BF16, kind="ExternalOutput").ap()

    with tile.TileContext(nc) as tc, ExitStack() as ctx:
        consts = ctx.enter_context(tc.tile_pool(name="consts", bufs=1))
        xt_pool = ctx.enter_context(tc.tile_pool(name="xt", bufs=n_dc))
        qkv_pool = ctx.enter_context(tc.tile_pool(name="qkv", bufs=2))
        vtil_pool = ctx.enter_context(tc.tile_pool(name="vtil", bufs=2 * hpc))
        p_pool = ctx.enter_context(tc.tile_pool(name="p", bufs=4))
        ot_pool = ctx.enter_context(tc.tile_pool(name="ot", bufs=2))
        ysb_pool = ctx.enter_context(tc.tile_pool(name="ysb", bufs=2))
        small_pool = ctx.enter_context(tc.tile_pool(name="small", bufs=2))

        ps_proj = ctx.enter_context(tc.tile_pool(name="psp", bufs=2, space="PSUM"))
        ps_s = ctx.enter_context(tc.tile_pool(name="pss", bufs=2, space="PSUM"))
        ps_av = ctx.enter_context(tc.tile_pool(name="psav", bufs=2, space="PSUM"))

        # --- constants ---
        wq_sb = consts.tile([128, n_dc, mh], BF16, tag="wq")
        wk_sb = consts.tile([128, n_dc, mh], BF16, tag="wk")
        wv_sb = consts.tile([128, n_dc, mh], BF16, tag="wv")
        for w_sb, w_dram in ((wq_sb, wq), (wk_sb, wk), (wv_sb, wv)):
            nc.sync.dma_start(w_sb[:], w_dram.rearrange("(c p) m -> p c m", p=128))
        woT_sb = consts.tile([mh, d], BF16, tag="wo")
        nc.sync.dma_start(woT_sb[:], woT[:])
        masks_sb = consts.tile([KC, 2, 2 * QB], BF16, tag="masks")
        nc.sync.dma_start(masks_sb[:], masks.rearrange("d p f -> p d f"))
        ident_sb = consts.tile([128, 64], F32, tag="ident")
        nc.sync.dma_start(ident_sb[:], ident[:])
        ones_bf = consts.tile([128, 1], BF16, tag="ones_bf")
        nc.vector.memset(ones_bf[:], 1.0)

        def make_proj_units(bi, st):
            """Micro-units (closures) for batch bi's loads + QKV projections
            + Vtilde, to be dribbled between attention quanta."""
            units = []

            def u_alloc():
                st["xt"] = []
                for c in range(n_dc):
                    xc = xt_pool.tile([128, t], BF16, tag="xt")
                    nc.sync.dma_start(xc[:], xT[bi, c * 128:(c + 1) * 128, :])
                    st["xt"].append(xc)
                st["qt2"] = qkv_pool.tile([mh, t], BF16, tag="qt2", name="qt2")
                st["kt2"] = qkv_pool.tile([mh, t], BF16, tag="kt2", name="kt2")
                st["vt2"] = qkv_pool.tile([mh, t], F32, tag="vt2", name="vt2")
            units.append(u_alloc)

            for key, wname in (("qt2", "wq"), ("kt2", "wk"), ("vt2", "wv")):
                for nb in range(n_qb):
                    def u_mm_a(key=key, wname=wname, nb=nb):
                        w_sb = {"wq": wq_sb, "wk": wk_sb, "wv": wv_sb}[wname]
                        acc = ps_proj.tile([mh, QB], F32, tag="proj")
                        st[f"acc_{key}_{nb}"] = acc
                        for c in range(n_dc // 2):
                            nc.tensor.matmul(
                                acc[:], w_sb[:, c, :],
                                st["xt"][c][:, nb * QB:(nb + 1) * QB],
                                start=(c == 0), stop=False)

                    def u_mm_b(key=key, wname=wname, nb=nb):
                        w_sb = {"wq": wq_sb, "wk": wk_sb, "wv": wv_sb}[wname]
                        acc = st[f"acc_{key}_{nb}"]
                        dst = st[key]
                        for c in range(n_dc // 2, n_dc):
                            nc.tensor.matmul(
                                acc[:], w_sb[:, c, :],
                                st["xt"][c][:, nb * QB:(nb + 1) * QB],
                                start=False, stop=(c == n_dc - 1))
                        if key == "kt2":
                            nc.scalar.copy(dst[:, nb * QB:(nb + 1) * QB],
                                           acc[:])
                        else:
                            nc.vector.tensor_copy(
                                dst[:, nb * QB:(nb + 1) * QB], acc[:])
                    units.append(u_mm_a)
                    units.append(u_mm_b)

            for hh in range(hpc):
                def u_vt_alloc(hh=hh):
                    vt = vtil_pool.tile([128, n_kc, 65], BF16, tag="vtil")
                    nc.vector.tensor_copy(
                        vt[:, :, 64], ones_bf[:].broadcast_to([128, n_kc]))
                    st[f"vtil{hh}"] = vt
                units.append(u_vt_alloc)
                for g in range(0, n_kc, 8):
                    def u_vtil(hh=hh, g=g):
                        vt = st[f"vtil{hh}"]
                        vt2 = st["vt2"]
                        gn = min(8, n_kc - g)
                        trp = ps_proj.tile([128, 512], F32, tag="proj")
                        for jj in range(gn):
                            j = g + jj
                            nc.tensor.transpose(
                                trp[:, jj * 64:(jj + 1) * 64],
                                vt2[hh * 64:(hh + 1) * 64,
                                    j * KC:(j + 1) * KC],
                                ident_sb[hh * 64:(hh + 1) * 64, :])
                        nc.vector.tensor_copy(
                            vt[:, g:g + gn, 0:64],
                            trp[:, 0:gn * 64].rearrange(
                                "p (j f) -> p j f", j=gn))
                    units.append(u_vtil)
            return units

        def make_attn_quanta(bi, st):
            """Attention for batch bi as a list of emission quanta, with one
            pair of S/exp lookahead ahead of each AV, and output-projection
            quanta deferred `defer_out` quanta into the following block."""
            quanta = []
            pending_out = []

            def q_pair(hh, qb, kc2, first, last):
                def emit():
                    qt2, kt2 = st["qt2"], st["kt2"]
                    qth = qt2[hh * 64:(hh + 1) * 64, :]
                    kth = kt2[hh * 64:(hh + 1) * 64, :]
                    kmax = (qb + 1) * (QB // KC)
                    if first:
                        st[f"oacc{hh}"] = ps_av.tile([128, QB], F32, tag="av", name="oacc")
                    oacc = st[f"oacc{hh}"]
                    vtil = st[f"vtil{hh}"]
                    # S pair + exp (+mask) for pair kc2
                    sps = ps_s.tile([KC, 2 * QB], F32, tag="s")
                    for i in range(2):
                        kc = 2 * kc2 + i
                        nc.tensor.matmul(
                            sps[:, i * QB:(i + 1) * QB],
                            kth[:, kc * KC:(kc + 1) * KC],
                            qth[:, qb * QB:(qb + 1) * QB],
                            start=True, stop=True)
                    psb = p_pool.tile([KC, 2 * QB], BF16, tag="p")
                    nc.scalar.activation(psb[:], sps[:], EXP, scale=SCALE)
                    r = kc2 - 2 * qb
                    if r >= 0:
                        nc.vector.tensor_mul(psb[:], psb[:], masks_sb[:, r, :])

                    def av():
                        for i in range(2):
                            kc = 2 * kc2 + i
                            nc.tensor.matmul(
                                oacc[0:65, :], vtil[:, kc, :],
                                psb[:, i * QB:(i + 1) * QB],
                                start=(kc == 0), stop=(kc == kmax - 1))
                        if last:
                            q_norm(hh, qb, oacc)

                    # lookahead: run the previous pair's AV after this S/exp
                    prev = st.get("pending_av")
                    st["pending_av"] = av
                    if prev is not None:
                        prev()
                return emit

            def flush_av():
                prev = st.pop("pending_av", None)
                if prev is not None:
                    prev()

            def q_norm(hh, qb, oacc):
                ot_core = st["ot"]
                recf = small_pool.tile([1, QB], F32, tag="recf")
                nc.vector.reciprocal(recf[:], oacc[64:65, :])
                bcs = small_pool.tile([64, QB], F32, tag="bcs")
                nc.gpsimd.partition_broadcast(bcs[:], recf[:])
                nc.vector.tensor_mul(
                    ot_core[hh * 64:(hh + 1) * 64, qb * QB:(qb + 1) * QB],
                    oacc[0:64, :], bcs[:])

            def q_out(tcn):
                def emit():
                    ot_core = st["ot"]
                    ysb = ysb_pool.tile([128, d], BF16, tag="ysb")
                    for nb0 in range(0, d, QB):
                        op = ps_proj.tile([128, QB], F32, tag="proj")
                        nc.tensor.matmul(
                            op[:], ot_core[:, tcn * 128:(tcn + 1) * 128],
                            woT_sb[:, nb0:nb0 + QB],
                            start=True, stop=True)
                        if (tcn + nb0 // QB) % 2 == 0:
                            nc.vector.tensor_copy(ysb[:, nb0:nb0 + QB], op[:])
                        else:
                            nc.scalar.copy(ysb[:, nb0:nb0 + QB], op[:])
                    nc.sync.dma_start(y[bi, tcn * 128:(tcn + 1) * 128, :],
                                      ysb[:])
                return emit

            def q_alloc_ot():
                st["ot"] = ot_pool.tile([mh, t], BF16, tag="ot", name="ot")

            for qb in range(n_qb):
                group = []
                if qb == 0:
                    group.append(q_alloc_ot)
                kmax = (qb + 1) * (QB // KC)
                for hh in range(hpc):
                    npair = kmax // 2
                    for kc2 in range(npair):
                        group.append(q_pair(hh, qb, kc2, kc2 == 0,
                                            kc2 == npair - 1))
                # interleave deferred outproj of the previous block
                out_slot = defer_out
                for j, g in enumerate(group):
                    quanta.append(g)
                    if j >= out_slot and pending_out:
                        quanta.append(pending_out.pop(0))
                        out_slot = j + 2
                pending_out.extend(
                    q_out(tcn) for tcn in range(qb * (QB // 128),
                                                (qb + 1) * (QB // 128)))
            quanta.append(flush_av)
            quanta.extend(pending_out)
            return quanta

        def run_quanta(quanta, fills):
            """Emit attention quanta; after quantum j emit fill units up to
            proportional pace (finishing at `pace` of the quanta list)."""
            emitted = 0
            nq = len(quanta)
            for j, q in enumerate(quanta):
                q()
                if fills:
                    want = min(len(fills),
                               int(len(fills) * (j + 1) / (pace * nq)))
                    while emitted < want:
                        fills[emitted]()
                        emitted += 1
            while emitted < len(fills):
                fills[emitted]()
                emitted += 1

        def body():
            states = [dict() for _ in range(b)]
            prev_quanta = None
            for bi in range(b):
                p_units = make_proj_units(bi, states[bi])
                if prev_quanta is None:
                    for u in p_units:
                        u()
                else:
                    run_quanta(prev_quanta, p_units)
                prev_quanta = make_attn_quanta(bi, states[bi])
            run_quanta(prev_quanta, [])

        if loop_n > 1:
            with tc.For_i(0, loop_n, 1):
                body()
        else:
            body()

    nc.compile()
    return nc


_NC_CACHE = {}


def _get_nc():
    if "nc" not in _NC_CACHE:
        _NC_CACHE["nc"] = build_nc()
    return _NC_CACHE["nc"]


def make_masks() -> np.ndarray:
    """Two paired masks [KC, 2*QB]: pair 0 = [delta 0 | delta 128],
    pair 1 = [delta 256 | delta 384]."""
    m = np.zeros((2, KC, 2 * QB), np.float32)
    p = np.arange(KC)[:, None]
    f = np.arange(QB)[None, :]
    for pair in range(2):
        for half in range(2):
            dlt = (2 * pair + half) * KC
            m[pair][:, half * QB:(half + 1) * QB] = \
                (p + dlt <= f).astype(np.float32)
    return m


def make_in_maps(x, Wq, Wk, Wv, Wo):
    import ml_dtypes
    bf = ml_dtypes.bfloat16
    xTr = np.ascontiguousarray(x.transpose(0, 2, 1)).astype(bf)
    masks = make_masks().astype(bf)
    ident = np.tile(np.eye(64, dtype=np.float32), (2, 1))
    in_maps = []
    for c in range(N_CORES):
        h0 = c * HPC
        wq2 = Wq[h0:h0 + HPC].transpose(1, 0, 2).reshape(D, 64 * HPC).astype(bf)
        wk2 = Wk[h0:h0 + HPC].transpose(1, 0, 2).reshape(D, 64 * HPC).astype(bf)
        wv2 = Wv[h0:h0 + HPC].transpose(1, 0, 2).reshape(D, 64 * HPC).astype(bf)
        woT = np.ascontiguousarray(
            Wo[:, h0 * 64:(h0 + HPC) * 64].T).astype(bf)
        in_maps.append({
            "xT": xTr, "wq": wq2, "wk": wk2, "wv": wv2, "woT": woT,
            "masks": masks, "ident": ident,
        })
    return in_maps


def kernel(x, Wq, Wk, Wv, Wo, bo):
    from concourse.bass_utils import run_bass_kernel_spmd

    x = np.asarray(x, np.float32)
    in_maps = make_in_maps(x, np.asarray(Wq, np.float32),
                           np.asarray(Wk, np.float32),
                           np.asarray(Wv, np.float32),
                           np.asarray(Wo, np.float32))
    nc = _get_nc()
    res = run_bass_kernel_spmd(nc, in_maps, core_ids=list(range(N_CORES)))
    out = res.results[0]["y"].astype(np.float64)
    for c in range(1, N_CORES):
        out += res.results[c]["y"].astype(np.float64)
    out += np.asarray(bo, np.float64)
    return out.astype(np.float32)
